# revision 20
# baseline (speedup 1.0000x reference)
"""Causal attention (B=8, N=4096, D=64) on 8 trn2 NeuronCores.

Sharding: batch b -> core b (data parallel, no cross-core comms).

Per-core kernel (flash-attention style, fully transposed dataflow -- no
on-chip transposes anywhere):
  inputs (host pre-layouts, fp16):
    qk    [64, nqb, 2, 512]  packed (kT | qT) chunks, d on partitions
    v_aug [128, N/128, 65]   k-tiled; col 64 = 1.0; padding-masked rows = 0
    tri   [128, 128]         triangular 0/1 keep-mask (y >= x)
  for each q-block (512 wide), k-tiles batched in TRIOS (3 x 128 keys):
    logitsT[k, q] = matmul(lhsT=kT_t [64,128], rhs=qT_blk [64,512])  (PSUM)
      -- diagonal tiles (j = t - 4*qb > 0) only compute live columns
         [128*j, 512), skipping the dead triangle region (~17% of PE work)
    expT = exp(logitsT_trio / sqrt(d))  ONE ACT op over [128,<=1536] -> SBUF
      -- trio batching amortizes the ~172-cycle ACT per-instruction
         overhead; large-offset diagonal tiles (j3,j2,j1) head the first
         batches of each block so the exp suffix-slice skips their dead
         prefix: 52 ACTIVATEs = 70.3us ACT-busy (the warm-run floor).
         PSUM caps the batch: lg 2x3 banks + acc 2x1 = 8 banks.
    diagonal tiles: 128-wide boundary strip *= tri mask             (DVE)
    outT[d,q] (+)= matmul(lhsT=v_aug [128,65], rhs=expT[:,c0:512])  (PSUM)
      -- v_aug col 64 is 1.0 => outT row 64 = the softmax denominators
  The MM2s are emitted TWO trios behind the MM1s/exp (PE stream
  [.. MM1s(b) MM2s(b-2) ..]) so exp+mask latency is fully hidden and the
  MM2 group head never waits on the scalar engine.
  Per q-block epilogue: acc is EVICTED to SBUF immediately (2 DVE copies,
  ~1.3us) so the PSUM bank recycles independent of any DMA backlog; then
  r = 1/sums via the fast custom-DVE reciprocal (reciprocal_approx_fast,
  ~5x faster than the iterative divide; its input is staged to a
  partition-0 SBUF tile because the custom op mis-reads partition-offset
  sources); broadcast r across partitions via a DRAM round-trip
  (partition-step-0 reads are DRAM-only); out = accT * r (DVE); DMA out.
  The last q-block uses a PE outer-product broadcast (shorter tail).
  Host transposes outT_dram [64, N] back to [N, 64] at gather time.

Input DMAs are interleaved across the sync and gpsimd queues (~49 GB/s
each), ordered by need-by time, so block 0 computes ~10us in and no
later block waits on its qk/v chunk.

Padding mask: host zeroes masked k rows of v_aug (incl. the ones column),
so masked keys contribute nothing to numerator or denominator -- exactly
equivalent to -inf logits.

Matmul operands are fp16 (1 cycle/row on the PE; fp32 PSUM accumulation);
measured rel err vs the fp32 reference is ~4e-4. Performance depends on
when the firmware grants the K=8/8 PE clock (2.4 GHz; grant lands 32-52us
into the run and drifts with chip temperature): best measured 98.4-99.1us,
typical 99-127us, ~150us if the power cap holds the PE at 1.2 GHz the
whole run. Warm steady state is ACT-paced at ~1.62us per trio (exp 1423ns
+ ~200ns scalar-queue semaphores); MMs stream back-to-back at 215-226ns
with weight loads fully hidden. Baseline measured 143us under the same
conditions that give this kernel ~110us.
"""

import os
from contextlib import ExitStack

import numpy as np

B, N, D = 8, 4096, 64
QBLK = 512
KTILE = 128

LAST_RESULTS = None
_NC_CACHE = {}


def build(n=N, d=D, qblk=QBLK, ktile=KTILE, batch=3, acc_bufs=2, pb_bufs=6,
          op_dt="float16", restrict=True, split_dma=True):
    import concourse.bass as bass
    import concourse.mybir as mybir
    import concourse.tile as tile
    from concourse import bacc

    f32 = mybir.dt.float32
    opd = getattr(mybir.dt, op_dt)   # matmul operand dtype
    qblk = min(qblk, n)
    nt = n // ktile          # number of k-tiles
    nqb = n // qblk          # number of q-blocks
    tpq = qblk // ktile      # k-tiles per q-block (diagonal span)

    nc = bacc.Bacc("TRN2", target_bir_lowering=False, debug=False,
                   enable_asserts=False)

    qk_d = nc.dram_tensor("qk", (d, nqb, 2, qblk), opd,
                          kind="ExternalInput").ap()
    v_d = nc.dram_tensor("v_aug", (128, nt, d + 1), opd,
                         kind="ExternalInput").ap()
    tri_d = nc.dram_tensor("tri", (128, ktile), opd,
                           kind="ExternalInput").ap()
    oT_d = nc.dram_tensor("outT", (d, n), f32, kind="ExternalOutput").ap()
    rs_d = nc.dram_tensor("rs_scratch", (nqb, qblk), f32,
                          kind="Internal").ap()

    scale = 1.0 / float(np.sqrt(d))

    with tile.TileContext(nc) as tc:
        with ExitStack() as ctx:
            singles = ctx.enter_context(tc.tile_pool(name="singles", bufs=1))
            pb_pool = ctx.enter_context(tc.tile_pool(name="pb", bufs=pb_bufs))
            small = ctx.enter_context(tc.tile_pool(name="small", bufs=3))
            ob_pool = ctx.enter_context(tc.tile_pool(name="ob", bufs=6))
            lg_pool = ctx.enter_context(
                tc.tile_pool(name="lg", bufs=2, space="PSUM"))
            acc_pool = ctx.enter_context(
                tc.tile_pool(name="acc", bufs=acc_bufs, space="PSUM"))

            # --- resident inputs -------------------------------------------
            qk_sb = singles.tile([d, nqb, 2, qblk], opd)
            v_sb = singles.tile([128, nt, d + 1], opd)
            tri_sb = singles.tile([128, ktile], opd)

            # Input DMAs interleaved across two queues (~49 GB/s each),
            # ordered by need-by time: block qb needs qk[:,qb] and
            # v[:,4qb:4qb+4] when it starts (ACT-paced ~1.4us per trio).
            q2 = nc.gpsimd if split_dma else nc.sync
            def qk_dma(q, c, ce):
                q.dma_start(out=qk_sb[:, c:ce], in_=qk_d[:, c:ce])
            def v_dma(q, c, ce):
                q.dma_start(out=v_sb[:, c * tpq:ce * tpq, :],
                            in_=v_d[:, c * tpq:ce * tpq, :])
            nc.sync.dma_start(out=qk_sb[:, 0, 0:1, :], in_=qk_d[:, 0, 0:1, :])
            q2.dma_start(out=qk_sb[:, 0, 1:2, :], in_=qk_d[:, 0, 1:2, :])
            q2.dma_start(out=tri_sb, in_=tri_d)
            v_dma(nc.sync, 0, 1)
            qk_dma(nc.sync, 1, 2)
            qk_dma(q2, 2, 3)
            v_dma(nc.sync, 1, 2)
            v_dma(q2, 2, 3)
            qk_dma(nc.sync, 3, 4)
            qk_dma(q2, 4, 5)
            v_dma(nc.sync, 3, 4)
            v_dma(q2, 4, 5)
            qk_dma(nc.sync, 5, 6)
            qk_dma(q2, 6, 7)
            v_dma(nc.sync, 5, 6)
            v_dma(q2, 6, 7)
            qk_dma(nc.sync, 7, 8)
            v_dma(q2, 7, 8)

            # First use of each lg buf: ACT reads the full trio width but
            # restricted MM1s leave dead columns unwritten -- zero them once
            # so the first exp never reads uninitialized PSUM.
            if restrict:
                for _ in range(2):
                    t0_ = lg_pool.tile([128, batch * qblk], f32, name="lginit",
                                       tag="lg")
                    nc.vector.memset(t0_, 0.0)

            def kT_ap(t):
                c, r = divmod(t, tpq)
                return qk_sb[:, c, 0, r * ktile:(r + 1) * ktile]

            # --- main loop -------------------------------------------------
            def epilogue(acc, qs, qb, last=False):
                # normalize: out = outT[0:64] / sums (sums = row d of acc).
                # Evict acc to SBUF immediately (two DVE copies) so the PSUM
                # bank recycles in ~1.3us -- the rest of the chain can then
                # ride out any DMA-queue backlog without stalling the PE.
                # The sums row is staged to partition 0 because the custom
                # DVE reciprocal mis-reads partition-offset sources.
                if not last:   # (the final block's acc is never reused)
                    accT = ob_pool.tile([d, qblk], f32, name="accT")
                    nc.vector.tensor_copy(accT, acc[0:d, :])
                ssum = small.tile([1, qblk], f32, name="ssum")
                nc.vector.tensor_copy(ssum, acc[d:d + 1, :])
                rsum = small.tile([1, qblk], f32, name="rsum")
                nc.vector.reciprocal_approx_fast(rsum, ssum)
                if not last:
                    nc.sync.dma_start(out=rs_d[qb:qb + 1, :], in_=rsum)
                    rb = ob_pool.tile([d, qblk], f32, name="rb")
                    rs_slice = rs_d[qb:qb + 1, :]
                    brd = bass.AP(tensor=rs_slice.tensor,
                                  offset=rs_slice.offset,
                                  ap=[[0, d], list(rs_slice.ap[-1])])
                    nc.sync.dma_start(out=rb, in_=brd)
                    ob = ob_pool.tile([d, qblk], f32, name="ob")
                    nc.vector.tensor_mul(ob, accT, rb)
                    nc.sync.dma_start(out=oT_d[:, qs:qs + qblk], in_=ob)
                    return
                # last q-block: PE outer-product broadcast (short tail chain)
                ones_f32 = small.tile([1, d], f32, name="ones_f32")
                nc.scalar.activation(
                    ones_f32, tri_sb[0:1, 0:d],
                    mybir.ActivationFunctionType.Copy)
                bc = lg_pool.tile([d, qblk], f32, name="bc", tag="lg")
                nc.tensor.matmul(bc, lhsT=ones_f32, rhs=rsum,
                                 start=True, stop=True)
                bc_sb = ob_pool.tile([d, qblk], f32, name="rb")
                nc.vector.tensor_copy(bc_sb, bc)
                ob = ob_pool.tile([d, qblk], f32, name="ob")
                nc.vector.tensor_mul(ob, acc[0:d, :], bc_sb)
                nc.sync.dma_start(out=oT_d[:, qs:qs + qblk], in_=ob)

            # Per trio: emit MM1s + exp + boundary-strip masks, then the
            # deferred MM2s from TWO trios back, so the PE stream
            # interleaves [... MM1s(b) MM2s(b-2) ...] and hides exp latency.
            # Tiles are reordered so the large-offset diagonal tiles (j=3,
            # j=2, j=1) head the first batches of each block: lg/pb are flat
            # [128, 3*512] so the exp can then skip the head's dead prefix
            # with a contiguous suffix slice (~4us less ACT work).
            mm2_q = []   # (acc, pb, tiles, c0s, qb, first_b, last_b)

            def flush_mm2():
                acc_, pb_, tiles_, c0s_, qb_, first_b, last_b = mm2_q.pop(0)
                order = list(range(len(tiles_)))
                if first_b:
                    # a full-width MM2 must come first: start=True resets
                    # PSUM has_written only for the columns it writes
                    order.sort(key=lambda i: c0s_[i])
                    assert c0s_[order[0]] == 0
                for n_, i in enumerate(order):
                    t, c0 = tiles_[i], c0s_[i]
                    nc.tensor.matmul(
                        acc_[:, c0:],
                        lhsT=v_sb[:, t, :],
                        rhs=pb_[:, i * qblk + c0:(i + 1) * qblk],
                        start=(first_b and n_ == 0),
                        stop=(last_b and n_ == len(order) - 1),
                        skip_group_check=True,
                    )
                if last_b:   # block done: normalize
                    epilogue(acc_, qb_ * qblk, qb_, last=(qb_ == nqb - 1))

            for qb in range(nqb):
                q_sl = qk_sb[:, qb, 1, :]
                acc = acc_pool.tile([d + 1, qblk], f32, name="acc", tag="acc")
                T = tpq * (qb + 1)
                if qb == 0:   # all-diagonal block: tile 0 must lead
                    seq = list(range(T))
                else:         # diag tiles j3,j2,j1,j0 head the first batches
                    offs = list(range(T - tpq))
                    diag = list(range(T - 1, T - tpq - 1, -1))
                    seq = []
                    while offs or diag:
                        if diag:
                            seq.append(diag.pop(0))
                        take = min(batch - 1, len(offs)) if diag or offs                             else 0
                        seq.extend(offs[:take])
                        offs = offs[take:]
                        if not diag and offs:
                            seq.extend(offs)
                            offs = []
                batches = [seq[s:s + batch] for s in range(0, T, batch)]
                for bi, tiles in enumerate(batches):
                    lg = lg_pool.tile([128, batch * qblk], f32, name="lg",
                                      tag="lg")
                    pb = pb_pool.tile([128, batch * qblk], opd, name="pb")
                    c0s = []
                    for i, t in enumerate(tiles):
                        j = t - tpq * qb
                        c0 = ktile * j if (restrict and j > 0) else 0
                        c0s.append(c0)
                        nc.tensor.matmul(
                            lg[:, i * qblk + c0:(i + 1) * qblk],
                            lhsT=kT_ap(t),
                            rhs=q_sl[:, c0:],
                            start=True, stop=True,
                        )
                    bs = len(tiles)
                    nc.scalar.activation(
                        pb[:, c0s[0]:bs * qblk], lg[:, c0s[0]:bs * qblk],
                        mybir.ActivationFunctionType.Exp, scale=scale)
                    for i, t in enumerate(tiles):
                        j = t - tpq * qb
                        if j >= 0:   # triangular boundary strip only
                            sl = pb[:, i * qblk + ktile * j:
                                    i * qblk + ktile * (j + 1)]
                            nc.vector.tensor_mul(sl, sl, tri_sb)
                    mm2_q.append((acc, pb, tiles, c0s, qb, bi == 0,
                                  bi == len(batches) - 1))
                    if len(mm2_q) >= 3:
                        flush_mm2()
            while mm2_q:
                flush_mm2()

    nc.compile()
    return nc


def _get_nc(key="main", **kw):
    if key not in _NC_CACHE:
        _NC_CACHE[key] = build(**kw)
    return _NC_CACHE[key]


def _prep_core_inputs(q, k, v, attn_mask, b, n=N, d=D, ktile=KTILE,
                      qblk=QBLK, op_dt="float16"):
    npdt = np.float16 if op_dt == "float16" else np.float32
    qblk = min(qblk, n)
    nt = n // ktile
    nqb = n // qblk
    qT = q[b].T.astype(npdt)          # [d, n]
    kT = k[b].T.astype(npdt)
    qk = np.empty((d, nqb, 2, qblk), dtype=npdt)
    qk[:, :, 0, :] = kT.reshape(d, nqb, qblk)
    qk[:, :, 1, :] = qT.reshape(d, nqb, qblk)
    v_aug = np.ones((n, d + 1), dtype=np.float32)
    v_aug[:, :d] = v[b]
    v_aug *= (attn_mask[b] != 0).astype(np.float32)[:, None]
    v_aug = np.ascontiguousarray(
        v_aug.reshape(nt, ktile, d + 1).transpose(1, 0, 2)).astype(npdt)
    # triangular 0/1 keep-mask for the 128-wide diagonal boundary strip
    y = np.arange(ktile)[None, :]
    x = np.arange(ktile)[:, None]
    tri = (y - x >= 0).astype(npdt)
    return {"qk": qk, "v_aug": v_aug, "tri": tri}


def kernel(q, k, v, attn_mask):
    global LAST_RESULTS
    q = np.asarray(q, dtype=np.float32)
    k = np.asarray(k, dtype=np.float32)
    v = np.asarray(v, dtype=np.float32)
    attn_mask = np.asarray(attn_mask)

    from concourse.bass_utils import run_bass_kernel_spmd

    nc = _get_nc()
    in_maps = [_prep_core_inputs(q, k, v, attn_mask, b) for b in range(B)]
    trace = bool(os.environ.get("BASS_TRACE"))
    last_err = None
    for attempt in range(3):
        try:
            LAST_RESULTS = run_bass_kernel_spmd(
                nc, in_maps, core_ids=list(range(B)), trace=trace)
            break
        except Exception as e:  # transient device-unrecoverable states clear
            last_err = e        # on the next execution attempt
            if "UNAVAILABLE" not in str(e) and "unrecoverable" not in str(e):
                raise
            import time as _time

            _time.sleep(2.0)
    else:
        raise last_err

    out = np.empty((B, N, D), dtype=np.float32)
    for b in range(B):
        out[b] = LAST_RESULTS.results[b]["outT"].T
    return out


# revision 21
# speedup vs baseline: 1.1771x; 1.1771x over previous
"""Causal attention (B=8, N=4096, D=64) on 8 trn2 NeuronCores.

Sharding: batch b -> core b (data parallel, no cross-core comms).

Per-core kernel (flash-attention style, fully transposed dataflow -- no
on-chip transposes anywhere):
  inputs (host pre-layouts, fp16):
    qk    [64, nqb, 2, 512]  packed (kT | qT) chunks, d on partitions
    v_aug [128, N/128, 65]   k-tiled; col 64 = 1.0; padding-masked rows = 0
    tri   [128, 128]         triangular 0/1 keep-mask (y >= x)
  for each q-block (512 wide), k-tiles batched in TRIOS (3 x 128 keys):
    logitsT[k, q] = matmul(lhsT=kT_t [64,128], rhs=qT_blk [64,512])  (PSUM)
      -- diagonal tiles (j = t - 4*qb > 0) only compute live columns
         [128*j, 512), skipping the dead triangle region (~17% of PE work)
    expT = exp(logitsT_trio / sqrt(d))  ONE ACT op over [128,<=1536] -> SBUF
      -- trio batching amortizes the ~172-cycle ACT per-instruction
         overhead; large-offset diagonal tiles (j3,j2,j1) head the first
         batches of each block so the exp suffix-slice skips their dead
         prefix: 52 ACTIVATEs = 70.3us ACT-busy (the warm-run floor).
         PSUM caps the batch: lg 2x3 banks + acc 2x1 = 8 banks.
    diagonal tiles: 128-wide boundary strip *= tri mask             (DVE)
    outT[d,q] (+)= matmul(lhsT=v_aug [128,65], rhs=expT[:,c0:512])  (PSUM)
      -- v_aug col 64 is 1.0 => outT row 64 = the softmax denominators
  The MM2s are emitted TWO trios behind the MM1s/exp (PE stream
  [.. MM1s(b) MM2s(b-2) ..]) so exp+mask latency is fully hidden and the
  MM2 group head never waits on the scalar engine.
  Per q-block epilogue: acc is EVICTED to SBUF immediately (2 DVE copies,
  ~1.3us) so the PSUM bank recycles independent of any DMA backlog; then
  r = 1/sums via the fast custom-DVE reciprocal (reciprocal_approx_fast,
  ~5x faster than the iterative divide; its input is staged to a
  partition-0 SBUF tile because the custom op mis-reads partition-offset
  sources); broadcast r across partitions via a DRAM round-trip
  (partition-step-0 reads are DRAM-only); out = accT * r (DVE); DMA out.
  The last q-block uses a PE outer-product broadcast (shorter tail).
  Host transposes outT_dram [64, N] back to [N, 64] at gather time.

Input DMAs are interleaved across the sync and gpsimd queues (~49 GB/s
each), ordered by need-by time, so block 0 computes ~10us in and no
later block waits on its qk/v chunk.

Padding mask: host zeroes masked k rows of v_aug (incl. the ones column),
so masked keys contribute nothing to numerator or denominator -- exactly
equivalent to -inf logits.

Matmul operands are fp16 (1 cycle/row on the PE; fp32 PSUM accumulation);
measured rel err vs the fp32 reference is ~4e-4. Performance depends on
when the firmware grants the K=8/8 PE clock (2.4 GHz; grant lands 32-52us
into the run and drifts with chip temperature): best measured 98.4-99.1us,
typical 99-127us, ~150us if the power cap holds the PE at 1.2 GHz the
whole run. Warm steady state is ACT-paced at ~1.62us per trio (exp 1423ns
+ ~200ns scalar-queue semaphores); MMs stream back-to-back at 215-226ns
with weight loads fully hidden. Baseline measured 143us under the same
conditions that give this kernel ~110us.
"""

import os
from contextlib import ExitStack

import numpy as np

B, N, D = 8, 4096, 64
QBLK = 512
KTILE = 128

LAST_RESULTS = None
_NC_CACHE = {}


def build(n=N, d=D, qblk=QBLK, ktile=KTILE, batch=3, acc_bufs=2, pb_bufs=24,
          op_dt="float16", restrict=True, split_dma=True):
    import concourse.bass as bass
    import concourse.mybir as mybir
    import concourse.tile as tile
    from concourse import bacc

    f32 = mybir.dt.float32
    opd = getattr(mybir.dt, op_dt)   # matmul operand dtype
    qblk = min(qblk, n)
    nt = n // ktile          # number of k-tiles
    nqb = n // qblk          # number of q-blocks
    tpq = qblk // ktile      # k-tiles per q-block (diagonal span)

    nc = bacc.Bacc("TRN2", target_bir_lowering=False, debug=False,
                   enable_asserts=False)

    qk_d = nc.dram_tensor("qk", (d, nqb, 2, qblk), opd,
                          kind="ExternalInput").ap()
    v_d = nc.dram_tensor("v_aug", (128, nt, d + 1), opd,
                         kind="ExternalInput").ap()
    tri_d = nc.dram_tensor("tri", (128, ktile), opd,
                           kind="ExternalInput").ap()
    oT_d = nc.dram_tensor("outT", (d, n), f32, kind="ExternalOutput").ap()
    rs_d = nc.dram_tensor("rs_scratch", (nqb, qblk), f32,
                          kind="Internal").ap()

    scale = 1.0 / float(np.sqrt(d))

    with tile.TileContext(nc) as tc:
        with ExitStack() as ctx:
            singles = ctx.enter_context(tc.tile_pool(name="singles", bufs=1))
            pb_pool = ctx.enter_context(tc.tile_pool(name="pb", bufs=pb_bufs))
            small = ctx.enter_context(tc.tile_pool(name="small", bufs=3))
            ob_pool = ctx.enter_context(tc.tile_pool(name="ob", bufs=6))
            lg_pool = ctx.enter_context(
                tc.tile_pool(name="lg", bufs=2, space="PSUM"))
            acc_pool = ctx.enter_context(
                tc.tile_pool(name="acc", bufs=acc_bufs, space="PSUM"))

            # --- resident inputs -------------------------------------------
            qk_sb = singles.tile([d, nqb, 2, qblk], opd)
            v_sb = singles.tile([128, nt, d + 1], opd)
            tri_sb = singles.tile([128, ktile], opd)

            # Input DMAs interleaved across two queues (~49 GB/s each),
            # ordered by need-by time: block qb needs qk[:,qb] and
            # v[:,4qb:4qb+4] when it starts (ACT-paced ~1.4us per trio).
            q2 = nc.gpsimd if split_dma else nc.sync
            def qk_dma(q, c, ce):
                q.dma_start(out=qk_sb[:, c:ce], in_=qk_d[:, c:ce])
            def v_dma(q, c, ce):
                q.dma_start(out=v_sb[:, c * tpq:ce * tpq, :],
                            in_=v_d[:, c * tpq:ce * tpq, :])
            nc.sync.dma_start(out=qk_sb[:, 0, 0:1, :], in_=qk_d[:, 0, 0:1, :])
            q2.dma_start(out=qk_sb[:, 0, 1:2, :], in_=qk_d[:, 0, 1:2, :])
            q2.dma_start(out=tri_sb, in_=tri_d)
            v_dma(nc.sync, 0, 1)
            qk_dma(nc.sync, 1, 2)
            qk_dma(q2, 2, 3)
            v_dma(nc.sync, 1, 2)
            v_dma(q2, 2, 3)
            qk_dma(nc.sync, 3, 4)
            qk_dma(q2, 4, 5)
            v_dma(nc.sync, 3, 4)
            v_dma(q2, 4, 5)
            qk_dma(nc.sync, 5, 6)
            qk_dma(q2, 6, 7)
            v_dma(nc.sync, 5, 6)
            v_dma(q2, 6, 7)
            qk_dma(nc.sync, 7, 8)
            v_dma(q2, 7, 8)

            # First use of each lg buf: ACT reads the full trio width but
            # restricted MM1s leave dead columns unwritten -- zero them once
            # so the first exp never reads uninitialized PSUM.
            if restrict:
                for _ in range(2):
                    t0_ = lg_pool.tile([128, batch * qblk], f32, name="lginit",
                                       tag="lg")
                    nc.vector.memset(t0_, 0.0)

            def kT_ap(t):
                c, r = divmod(t, tpq)
                return qk_sb[:, c, 0, r * ktile:(r + 1) * ktile]

            # --- main loop -------------------------------------------------
            def epilogue(acc, qs, qb, last=False):
                # normalize: out = outT[0:64] / sums (sums = row d of acc).
                # Evict acc to SBUF immediately (two DVE copies) so the PSUM
                # bank recycles in ~1.3us -- the rest of the chain can then
                # ride out any DMA-queue backlog without stalling the PE.
                # The sums row is staged to partition 0 because the custom
                # DVE reciprocal mis-reads partition-offset sources.
                if not last:   # (the final block's acc is never reused)
                    accT = ob_pool.tile([d, qblk], f32, name="accT")
                    nc.vector.tensor_copy(accT, acc[0:d, :])
                ssum = small.tile([1, qblk], f32, name="ssum")
                nc.vector.tensor_copy(ssum, acc[d:d + 1, :])
                rsum = small.tile([1, qblk], f32, name="rsum")
                nc.vector.reciprocal_approx_fast(rsum, ssum)
                if not last:
                    nc.sync.dma_start(out=rs_d[qb:qb + 1, :], in_=rsum)
                    rb = ob_pool.tile([d, qblk], f32, name="rb")
                    rs_slice = rs_d[qb:qb + 1, :]
                    brd = bass.AP(tensor=rs_slice.tensor,
                                  offset=rs_slice.offset,
                                  ap=[[0, d], list(rs_slice.ap[-1])])
                    nc.sync.dma_start(out=rb, in_=brd)
                    ob = ob_pool.tile([d, qblk], f32, name="ob")
                    nc.vector.tensor_mul(ob, accT, rb)
                    nc.sync.dma_start(out=oT_d[:, qs:qs + qblk], in_=ob)
                    return
                # last q-block: PE outer-product broadcast (short tail chain)
                ones_f32 = small.tile([1, d], f32, name="ones_f32")
                nc.scalar.activation(
                    ones_f32, tri_sb[0:1, 0:d],
                    mybir.ActivationFunctionType.Copy)
                bc = lg_pool.tile([d, qblk], f32, name="bc", tag="lg")
                nc.tensor.matmul(bc, lhsT=ones_f32, rhs=rsum,
                                 start=True, stop=True)
                bc_sb = ob_pool.tile([d, qblk], f32, name="rb")
                nc.vector.tensor_copy(bc_sb, bc)
                ob = ob_pool.tile([d, qblk], f32, name="ob")
                nc.vector.tensor_mul(ob, acc[0:d, :], bc_sb)
                nc.sync.dma_start(out=oT_d[:, qs:qs + qblk], in_=ob)

            # Per trio: emit MM1s + exp + boundary-strip masks, then the
            # deferred MM2s from TWO trios back, so the PE stream
            # interleaves [... MM1s(b) MM2s(b-2) ...] and hides exp latency.
            # Tiles are reordered so the large-offset diagonal tiles (j=3,
            # j=2, j=1) head the first batches of each block: lg/pb are flat
            # [128, 3*512] so the exp can then skip the head's dead prefix
            # with a contiguous suffix slice (~4us less ACT work).
            mm2_q = []   # (acc, pb, tiles, c0s, qb, first_b, last_b)

            def flush_mm2():
                acc_, pb_, tiles_, c0s_, qb_, first_b, last_b = mm2_q.pop(0)
                order = list(range(len(tiles_)))
                if first_b:
                    # a full-width MM2 must come first: start=True resets
                    # PSUM has_written only for the columns it writes
                    order.sort(key=lambda i: c0s_[i])
                    assert c0s_[order[0]] == 0
                for n_, i in enumerate(order):
                    t, c0 = tiles_[i], c0s_[i]
                    nc.tensor.matmul(
                        acc_[:, c0:],
                        lhsT=v_sb[:, t, :],
                        rhs=pb_[:, i * qblk + c0:(i + 1) * qblk],
                        start=(first_b and n_ == 0),
                        stop=(last_b and n_ == len(order) - 1),
                        skip_group_check=True,
                    )
                if last_b:   # block done: normalize
                    epilogue(acc_, qb_ * qblk, qb_, last=(qb_ == nqb - 1))

            for qb in range(nqb):
                q_sl = qk_sb[:, qb, 1, :]
                acc = acc_pool.tile([d + 1, qblk], f32, name="acc", tag="acc")
                T = tpq * (qb + 1)
                if qb == 0:   # all-diagonal block: tile 0 must lead
                    seq = list(range(T))
                else:         # diag tiles j3,j2,j1,j0 head the first batches
                    offs = list(range(T - tpq))
                    diag = list(range(T - 1, T - tpq - 1, -1))
                    seq = []
                    while offs or diag:
                        if diag:
                            seq.append(diag.pop(0))
                        take = min(batch - 1, len(offs)) if diag or offs                             else 0
                        seq.extend(offs[:take])
                        offs = offs[take:]
                        if not diag and offs:
                            seq.extend(offs)
                            offs = []
                batches = [seq[s:s + batch] for s in range(0, T, batch)]
                for bi, tiles in enumerate(batches):
                    lg = lg_pool.tile([128, batch * qblk], f32, name="lg",
                                      tag="lg")
                    pb = pb_pool.tile([128, batch * qblk], opd, name="pb")
                    c0s = []
                    for i, t in enumerate(tiles):
                        j = t - tpq * qb
                        c0 = ktile * j if (restrict and j > 0) else 0
                        c0s.append(c0)
                        nc.tensor.matmul(
                            lg[:, i * qblk + c0:(i + 1) * qblk],
                            lhsT=kT_ap(t),
                            rhs=q_sl[:, c0:],
                            start=True, stop=True,
                        )
                    bs = len(tiles)
                    nc.scalar.activation(
                        pb[:, c0s[0]:bs * qblk], lg[:, c0s[0]:bs * qblk],
                        mybir.ActivationFunctionType.Exp, scale=scale)
                    for i, t in enumerate(tiles):
                        j = t - tpq * qb
                        if j >= 0:   # triangular boundary strip only
                            sl = pb[:, i * qblk + ktile * j:
                                    i * qblk + ktile * (j + 1)]
                            nc.vector.tensor_mul(sl, sl, tri_sb)
                    mm2_q.append((acc, pb, tiles, c0s, qb, bi == 0,
                                  bi == len(batches) - 1))
                    if len(mm2_q) >= 3:
                        flush_mm2()
            while mm2_q:
                flush_mm2()

    nc.compile()
    return nc


def _get_nc(key="main", **kw):
    if key not in _NC_CACHE:
        _NC_CACHE[key] = build(**kw)
    return _NC_CACHE[key]


def _prep_core_inputs(q, k, v, attn_mask, b, n=N, d=D, ktile=KTILE,
                      qblk=QBLK, op_dt="float16"):
    npdt = np.float16 if op_dt == "float16" else np.float32
    qblk = min(qblk, n)
    nt = n // ktile
    nqb = n // qblk
    qT = q[b].T.astype(npdt)          # [d, n]
    kT = k[b].T.astype(npdt)
    qk = np.empty((d, nqb, 2, qblk), dtype=npdt)
    qk[:, :, 0, :] = kT.reshape(d, nqb, qblk)
    qk[:, :, 1, :] = qT.reshape(d, nqb, qblk)
    v_aug = np.ones((n, d + 1), dtype=np.float32)
    v_aug[:, :d] = v[b]
    v_aug *= (attn_mask[b] != 0).astype(np.float32)[:, None]
    v_aug = np.ascontiguousarray(
        v_aug.reshape(nt, ktile, d + 1).transpose(1, 0, 2)).astype(npdt)
    # triangular 0/1 keep-mask for the 128-wide diagonal boundary strip
    y = np.arange(ktile)[None, :]
    x = np.arange(ktile)[:, None]
    tri = (y - x >= 0).astype(npdt)
    return {"qk": qk, "v_aug": v_aug, "tri": tri}


def kernel(q, k, v, attn_mask):
    global LAST_RESULTS
    q = np.asarray(q, dtype=np.float32)
    k = np.asarray(k, dtype=np.float32)
    v = np.asarray(v, dtype=np.float32)
    attn_mask = np.asarray(attn_mask)

    from concourse.bass_utils import run_bass_kernel_spmd

    nc = _get_nc()
    in_maps = [_prep_core_inputs(q, k, v, attn_mask, b) for b in range(B)]
    trace = bool(os.environ.get("BASS_TRACE"))
    last_err = None
    for attempt in range(3):
        try:
            LAST_RESULTS = run_bass_kernel_spmd(
                nc, in_maps, core_ids=list(range(B)), trace=trace)
            break
        except Exception as e:  # transient device-unrecoverable states clear
            last_err = e        # on the next execution attempt
            if "UNAVAILABLE" not in str(e) and "unrecoverable" not in str(e):
                raise
            import time as _time

            _time.sleep(2.0)
    else:
        raise last_err

    out = np.empty((B, N, D), dtype=np.float32)
    for b in range(B):
        out[b] = LAST_RESULTS.results[b]["outT"].T
    return out
